# revision 7
# baseline (speedup 1.0000x reference)
"""Multi-head self-attention Trainium2 Bass kernel (v2: fused stream).

Problem: y = (softmax((x@Wq)(x@Wk)^T / sqrt(hd)) (x@Wv)) @ Wp + biases
with B=4, T=2048, C=1024, H=16, hd=64.

Sharding over 8 NeuronCores: (batch b in 0..3) x (head-group g in 0..1, 8
heads each).  Each core computes attention for its batch and head group
plus the partial output projection restricted to its head group's
features; the host sums the two head-group partials per batch.

v2 design (vs the chunked baseline): ONE fused PE instruction stream.
The attention inner loop (per key-block kc: 2 score matmuls -> one fused
two-head exp [128,1024] on ScalarE -> 2 AV matmuls) is ACT/PE balanced,
so projection matmuls (QK-proj, V-proj, out-proj) are interleaved as
"filler" work paced by a simple time model to keep the PE queue
always-ready: no PE stalls => the tensor engine stays at its top p-state
(2.4 GHz).  All matmul inputs are bf16 (halves DMA + SBUF vs f32).

PSUM layout (8 banks):
  sps0, sps1  [128,1024] 2 banks each -- score parities (kc%2), each
              holds both heads' scores for one 512-wide q window.
  u0a, u0b    [128, 512] 1 bank each -- head-0 AV accumulator, double-
              buffered across segments (hides the DVE normalize).
  u1          [128, 512] -- head-1 AV accumulator (normalized first).
  fb          [128, 512] -- filler accumulator (proj units).
"""

import os

import numpy as np
import ml_dtypes

import concourse.bass as bass
import concourse.bacc as bacc
import concourse.tile as tile
from concourse import mybir
from concourse.bass_utils import run_bass_kernel_spmd

N_CORES = 8
C = 1024           # embed dim
H = 16             # total heads
HD = 64            # head dim
HPC = 8            # heads per core
CG = HPC * HD      # 512: per-core q/k/v feature width
QW = 512           # q-window width per attention segment

F32 = mybir.dt.float32
BF16 = mybir.dt.bfloat16

# pacing model (ns), calibrated on HW -- drives filler insertion only
MM_NS = 330.0      # one N=512 bf16 matmul incl instruction-issue overhead
ACT_NS = 1950.0    # one [128,1024] exp from PSUM (measured ~2.0us)
DVE_NS = 2600.0    # one [128,512] DVE op reading PSUM (measured ~2.6-3.1us)
SEM_NS = 150.0


def _body(tc, T, x_t, w_qk, b_qk, w_v, ones_bf, w_p, b_out, out_t):
    nc = tc.nc
    KC = C // 128           # 8: contraction chunks over C
    TC1 = T // 128          # key blocks (kc loop trip count)
    T4 = T // 512           # 512-wide token windows
    NP = HPC // 2           # 4 head pairs
    PCH = CG // 128         # 4 out-proj contraction chunks
    OCC = C // 128          # 8 out-channel chunks
    Exp = mybir.ActivationFunctionType.Exp
    Mult = mybir.AluOpType.mult

    with (
        tc.tile_pool(name="pers", bufs=1) as pers,
        tc.tile_pool(name="et", bufs=3) as ep,
        tc.tile_pool(name="rec", bufs=2) as recp,
        tc.tile_pool(name="osb", bufs=4) as outp,
        tc.tile_pool(name="ps", bufs=1, space="PSUM") as psp,
    ):
        # ---- persistent SBUF; DMA issue order = arrival order, so
        # interleave xt/wv per contraction chunk (the V prologue consumes
        # them kc-major while the rest still streams in).
        bqk_sb = pers.tile([128, 2 * CG // 128], F32, tag="bqk")
        nc.sync.dma_start(bqk_sb[:], b_qk[:])
        bout_sb = pers.tile([128, OCC], F32, tag="bout")
        nc.sync.dma_start(bout_sb[:], b_out[:])

        xt, wv = [], []
        for kc in range(KC):
            t = pers.tile([128, T], BF16, tag=f"xt{kc}", name=f"xt{kc}")
            nc.sync.dma_start(t[:], x_t[kc * 128:(kc + 1) * 128, :])
            xt.append(t)
            t = pers.tile([128, CG], BF16, tag=f"wv{kc}", name=f"wv{kc}")
            nc.sync.dma_start(t[:], w_v[kc * 128:(kc + 1) * 128, :])
            wv.append(t)
        wqk = {}
        for fc in range(2 * CG // 128):
            for kc in range(KC):
                t = pers.tile([128, 128], BF16, tag=f"wqk_{fc}_{kc}",
                              name=f"wqk_{fc}_{kc}")
                nc.sync.dma_start(
                    t[:], w_qk[kc * 128:(kc + 1) * 128,
                               fc * 128:(fc + 1) * 128])
                wqk[(fc, kc)] = t
        wp = []
        for f in range(PCH):
            t = pers.tile([128, C], BF16, tag=f"wp{f}", name=f"wp{f}")
            nc.sync.dma_start(t[:], w_p[f * 128:(f + 1) * 128, :])
            wp.append(t)

        # per-head blocks of 128 cols: [V_h (64) | ones (64)] so one M=128
        # AV matmul yields U^T rows 0:64 and the replicated softmax
        # denominator rows 64:128.
        v2 = [pers.tile([128, 2 * CG], BF16, tag=f"v2_{i}", name=f"v2_{i}")
              for i in range(TC1)]
        qkt = [pers.tile([128, T], BF16, tag=f"qkt{f}", name=f"qkt{f}")
               for f in range(2 * CG // 128)]
        yt = [pers.tile([128, T], BF16, tag=f"yt{p}", name=f"yt{p}")
              for p in range(NP)]

        # ---- pacing model state (drives filler insertion only) ----
        st = {"pe": 0.0, "act": 0.0, "dve": 0.0, "uid": 0}

        def mm():
            st["pe"] += MM_NS

        def dve():
            st["dve"] = max(st["dve"], st["pe"] + SEM_NS) + DVE_NS

        def v_drain(tokc, ps):
            v2v = v2[tokc].rearrange("p (h c) -> p h c", c=2 * HD)
            nc.vector.tensor_copy(
                v2v[:, :, 0:HD], ps.rearrange("p (h c) -> p h c", c=HD))
            dve()
            nc.sync.dma_start(
                v2v[:, :, HD:2 * HD],
                ones_bf.rearrange("p (h c) -> p h c", c=HD))

        # ---- V-projection prologue: kc-major in batches of 4 token
        # blocks so the PE starts as soon as xt[0]/wv[0] land; batch b and
        # b+1 use disjoint psum banks so b's drains have a whole batch of
        # slack.
        slotsA = ("sps0", "sps1")        # sliced in halves -> 4 slots
        slotsB = ("u0a", "u0b", "u1", "fb")
        for b0 in range(0, TC1, 4):
            if (b0 // 4) % 2 == 0:
                g0 = psp.tile([128, 1024], F32, tag=slotsA[0],
                              name=f"vpro{b0}_0")
                g1 = psp.tile([128, 1024], F32, tag=slotsA[1],
                              name=f"vpro{b0}_1")
                slots = [g0[:, 0:512], g0[:, 512:1024],
                         g1[:, 0:512], g1[:, 512:1024]]
            else:
                slots = [psp.tile([128, 512], F32, tag=tg,
                                  name=f"vpro{b0}_{tg}") for tg in slotsB]
            n = min(4, TC1 - b0)
            for kc in range(KC):
                for j in range(n):
                    nc.tensor.matmul(
                        slots[j], xt[kc][:, (b0 + j) * 128:(b0 + j + 1) * 128],
                        wv[kc][:], start=(kc == 0), stop=(kc == KC - 1))
                    mm()
            for j in range(n):
                v_drain(b0 + j, slots[j])

        # ---- filler units: (name, mm-emitters factory, drain) ----
        def qk_unit(fc, t4):
            def mms(ps):
                out = []
                for kc in range(KC):
                    def e(kc=kc, ps=ps):
                        nc.tensor.matmul(
                            ps[:], wqk[(fc, kc)][:],
                            xt[kc][:, t4 * 512:(t4 + 1) * 512],
                            start=(kc == 0), stop=(kc == KC - 1))
                        mm()
                    out.append(e)
                return out

            def drain(ps):
                nc.vector.tensor_scalar_add(
                    qkt[fc][:, t4 * 512:(t4 + 1) * 512], ps[:],
                    bqk_sb[:, fc:fc + 1])
                dve()
            return (f"qk_{fc}_{t4}", mms, drain)

        def o_unit(occ, t4):
            def mms(ps):
                out = []
                for f in range(PCH):
                    def e(f=f, ps=ps):
                        nc.tensor.matmul(
                            ps[:], wp[f][:, occ * 128:(occ + 1) * 128],
                            yt[f][:, t4 * 512:(t4 + 1) * 512],
                            start=(f == 0), stop=(f == PCH - 1))
                        mm()
                    out.append(e)
                return out

            def drain(ps):
                osb = outp.tile([128, 512], F32, tag="osb",
                                name=f"osb_{occ}_{t4}")
                nc.vector.tensor_scalar_add(osb[:], ps[:],
                                            bout_sb[:, occ:occ + 1])
                dve()
                nc.sync.dma_start(
                    out_t[occ * 128:(occ + 1) * 128,
                          t4 * 512:(t4 + 1) * 512], osb[:])
            return (f"o_{occ}_{t4}", mms, drain)

        # filler machinery: deadline-ordered queue; emit_filler() emits ONE
        # matmul (psum slot allocated on first touch, DVE drain right after
        # the last matmul).
        queue = []
        cur = {"mms": None, "i": 0, "drain": None, "ps": None, "name": None}
        done_units = set()

        def emit_filler(slot_tags=("fb",)):
            if cur["mms"] is None:
                if not queue:
                    return False
                name, mmf, drain = queue.pop(0)
                tag = slot_tags[st["uid"] % len(slot_tags)]
                ps = psp.tile([128, 512], F32, tag=tag, name=f"acc_{name}")
                st["uid"] += 1
                cur.update(mms=mmf(ps), i=0, drain=drain, ps=ps, name=name)
            cur["mms"][cur["i"]]()
            cur["i"] += 1
            if cur["i"] >= len(cur["mms"]):
                cur["drain"](cur["ps"])
                done_units.add(cur["name"])
                cur["mms"] = None
            return True

        def emit_pad():
            # only when no unit is mid-accumulation in fb
            if cur["mms"] is not None:
                return emit_filler(("fb",))
            ps = psp.tile([128, 512], F32, tag="fb",
                          name=f"pad{st['uid']}")
            st["uid"] += 1
            nc.tensor.matmul(ps[:], wqk[(0, 0)][:], xt[0][:, 0:512],
                             start=True, stop=True)
            mm()
            return True

        def force(name, slot_tags=("fb",)):
            while name not in done_units:
                assert emit_filler(slot_tags), f"filler underflow at {name}"

        # ---- rest of prologue: K0, Q(0,0) first (their DVE drains settle
        # while K1/K2 stream), then K1, K2 for the first window group.
        for t4 in range(T4):
            queue.append(qk_unit(NP + 0, t4))
        queue.append(qk_unit(0, 0))
        for p in (1, 2):
            for t4 in range(T4):
                queue.append(qk_unit(NP + p, t4))
        while emit_filler(("fb", "u0a", "u0b", "u1")):
            pass

        # ---- stream queue for window group 0 ----
        queue.extend([qk_unit(1, 0), qk_unit(2, 0)])
        for t4 in range(T4):
            queue.append(qk_unit(NP + 3, t4))
        queue.append(qk_unit(3, 0))

        # ---- attention segments: window-outer (qc-outer), pair inner, so
        # out-proj for window w unlocks after only 4 segments.
        seg_idx = 0
        u1_free = 0.0
        for w in range(T4):
            q0 = w * QW
            for p in range(NP):
                force(f"qk_{p}_{w}")
                qt, kt = qkt[p], qkt[NP + p]
                u0_tag = ("u0a", "u0b")[seg_idx % 2]
                u0_other = ("u0a", "u0b")[(seg_idx + 1) % 2]
                u0 = psp.tile([128, 512], F32, tag=u0_tag,
                              name=f"u0_{p}_{w}")
                u1t = psp.tile([128, 512], F32, tag="u1", name=f"u1_{p}_{w}")
                ups = (u0, u1t)
                ets = {}
                act_done = {}

                def emit_S(kc):
                    sgen = psp.tile([128, 1024], F32, tag=f"sps{kc % 2}",
                                    name=f"sps_{p}_{w}_{kc}")
                    for s in (0, 1):
                        po = s * 64
                        nc.tensor.matmul(
                            sgen[:, s * 512:(s + 1) * 512],
                            kt[po:po + 64, kc * 128:(kc + 1) * 128],
                            qt[po:po + 64, q0:q0 + QW],
                            start=True, stop=True)
                        mm()
                    et = ep.tile([128, 1024], BF16, tag="et",
                                 name=f"et_{p}_{w}_{kc}")
                    if os.environ.get("KPROBE") == "exp2x":
                        nc.scalar.activation(et[:, 0:512], sgen[:, 0:512],
                                             Exp, scale=0.125)
                        nc.scalar.activation(et[:, 512:1024],
                                             sgen[:, 512:1024],
                                             Exp, scale=0.125)
                    else:
                        nc.scalar.activation(et[:], sgen[:], Exp, scale=0.125)
                    st["act"] = max(st["act"], st["pe"] + SEM_NS) + ACT_NS
                    act_done[kc] = st["act"]
                    ets[kc] = et

                def emit_A(kc):
                    et = ets.pop(kc)
                    for s in (0, 1):
                        nc.tensor.matmul(
                            ups[s][:],
                            v2[kc][:, (2 * p + s) * 128:
                                   (2 * p + s + 1) * 128],
                            et[:, s * 512:(s + 1) * 512],
                            start=(kc == 0), stop=(kc == TC1 - 1))
                        mm()

                # segment pipeline: S runs two kc ahead of A.  Interior
                # fillers may also use the idle u0 parity bank (their drain
                # finishes long before the next segment claims it).
                emit_S(0)
                emit_S(1)
                for kc in range(TC1):
                    slot_tags = (("fb", u0_other)
                                 if 2 <= kc < TC1 - 6 else ("fb",))
                    tgt = act_done[kc] + SEM_NS
                    if kc == 0:
                        tgt = max(tgt, u1_free + SEM_NS)
                    while st["pe"] < tgt - MM_NS:
                        if not emit_filler(slot_tags) and not emit_pad():
                            break
                    emit_A(kc)
                    if kc + 2 < TC1:
                        emit_S(kc + 2)

                # normalize: head 1 first (its accumulator is
                # single-buffered and blocks the next segment's A(0))
                for s in (1, 0):
                    po = s * 64
                    rec = recp.tile([64, QW], F32, tag="rec",
                                    name=f"rec_{p}_{w}_{s}")
                    nc.vector.reciprocal(rec[:], ups[s][64:128, :])
                    dve()
                    nc.vector.tensor_tensor(
                        yt[p][po:po + 64, q0:q0 + QW],
                        ups[s][0:64, :], rec[:], op=Mult)
                    if s == 1:
                        u1_free = st["dve"] + DVE_NS
                    dve()
                seg_idx += 1
            # group w done: queue next window's Q chunks, then the now-legal
            # out-proj units for window w
            if w + 1 < T4:
                for p in range(NP):
                    queue.append(qk_unit(p, w + 1))
            for occ in range(OCC):
                queue.append(o_unit(occ, w))

        # ---- epilogue: drain remaining fillers (leftover out-proj) ----
        while emit_filler(("fb", "u0a", "u0b", "u1")):
            pass


def build_nc(T=2048):
    FC = 2 * CG // 128
    OCC = C // 128
    nc = bacc.Bacc("TRN2", target_bir_lowering=False, debug=False,
                   num_devices=N_CORES)
    x_t = nc.dram_tensor("x_t", [C, T], BF16, kind="ExternalInput")
    w_qk = nc.dram_tensor("w_qk", [C, 2 * CG], BF16, kind="ExternalInput")
    b_qk = nc.dram_tensor("b_qk", [128, FC], F32, kind="ExternalInput")
    w_v = nc.dram_tensor("w_v", [C, CG], BF16, kind="ExternalInput")
    ones_bf = nc.dram_tensor("ones_bf", [128, CG], BF16, kind="ExternalInput")
    w_p = nc.dram_tensor("w_p", [CG, C], BF16, kind="ExternalInput")
    b_out = nc.dram_tensor("b_out", [128, OCC], F32, kind="ExternalInput")
    out_t = nc.dram_tensor("out_t", [C, T], F32, kind="ExternalOutput")
    with tile.TileContext(nc) as tc:
        _body(tc, T, x_t.ap(), w_qk.ap(), b_qk.ap(), w_v.ap(), ones_bf.ap(),
              w_p.ap(), b_out.ap(), out_t.ap())
    nc.compile()
    return nc


def shard_inputs(sequences, w_attn, b_attn, w_proj, b_proj):
    """Build the 8 per-core input maps. Core index = b*2 + g."""
    sequences = np.asarray(sequences, dtype=np.float32)
    w_attn = np.asarray(w_attn, dtype=np.float32)
    b_attn = np.asarray(b_attn, dtype=np.float32)
    w_proj = np.asarray(w_proj, dtype=np.float32)
    b_proj = np.asarray(b_proj, dtype=np.float32)
    B = sequences.shape[0]
    in_maps = []
    for b in range(B):
        for g in range(2):
            qs = slice(g * CG, (g + 1) * CG)
            ks = slice(C + g * CG, C + (g + 1) * CG)
            vs = slice(2 * C + g * CG, 2 * C + (g + 1) * CG)
            in_maps.append({
                "x_t": np.ascontiguousarray(sequences[b].T)
                    .astype(ml_dtypes.bfloat16),
                "w_qk": np.ascontiguousarray(
                    np.concatenate([w_attn[:, qs], w_attn[:, ks]], axis=1))
                    .astype(ml_dtypes.bfloat16),
                "b_qk": np.ascontiguousarray(
                    np.concatenate([b_attn[qs], b_attn[ks]])
                    .reshape(8, 128).T),
                "w_v": np.ascontiguousarray(w_attn[:, vs])
                    .astype(ml_dtypes.bfloat16),
                "ones_bf": np.ones((128, CG), ml_dtypes.bfloat16),
                "w_p": np.ascontiguousarray(w_proj[g * CG:(g + 1) * CG, :])
                    .astype(ml_dtypes.bfloat16),
                # softmax rows sum to 1, so the v-bias folds into the output
                # bias: y_g = attn@(x@w_v) @ w_p + (b_v@w_p [+ b_proj on g0])
                "b_out": np.ascontiguousarray(
                    (b_attn[vs] @ w_proj[g * CG:(g + 1) * CG, :]
                     + (b_proj if g == 0 else 0.0))
                    .astype(np.float32).reshape(8, 128).T),
            })
    return in_maps


def unshard_outputs(outs, B, T):
    """outs: list of 8 [C, T] partials, core index = b*2+g."""
    y = np.empty((B, T, C), np.float32)
    for b in range(B):
        y[b] = (outs[2 * b] + outs[2 * b + 1]).T
    return y


_NC_CACHE = {}


def kernel(sequences, w_attn, b_attn, w_proj, b_proj):
    sequences = np.asarray(sequences, dtype=np.float32)
    B, T, _ = sequences.shape
    in_maps = shard_inputs(sequences, w_attn, b_attn, w_proj, b_proj)
    if T not in _NC_CACHE:
        _NC_CACHE[T] = build_nc(T)
    nc = _NC_CACHE[T]
    res = run_bass_kernel_spmd(nc, in_maps, list(range(N_CORES)))
    outs = [res.results[i]["out_t"] for i in range(N_CORES)]
    return unshard_outputs(outs, B, T)


if __name__ == "__main__":
    rng = np.random.default_rng(0)
    B, T = 4, 2048
    seq = rng.standard_normal((B, T, C), dtype=np.float32)
    wa = rng.standard_normal((C, 3 * C), dtype=np.float32) / np.sqrt(C)
    ba = np.zeros(3 * C, np.float32)
    wp = rng.standard_normal((C, C), dtype=np.float32) / np.sqrt(C)
    bp = np.zeros(C, np.float32)
    y = kernel(seq, wa, ba, wp, bp)
    print(y.shape, y.dtype)


# revision 9
# speedup vs baseline: 1.0246x; 1.0246x over previous
"""Multi-head self-attention Trainium2 Bass kernel (v2: fused stream).

Problem: y = (softmax((x@Wq)(x@Wk)^T / sqrt(hd)) (x@Wv)) @ Wp + biases
with B=4, T=2048, C=1024, H=16, hd=64.

Sharding over 8 NeuronCores: (batch b in 0..3) x (head-group g in 0..1, 8
heads each).  Each core computes attention for its batch and head group
plus the partial output projection restricted to its head group's
features; the host sums the two head-group partials per batch.

v2 design (vs the chunked baseline): ONE fused PE instruction stream.
The attention inner loop (per key-block kc: 2 score matmuls -> one fused
two-head exp [128,1024] on ScalarE -> 2 AV matmuls) is ACT/PE balanced,
so projection matmuls (QK-proj, V-proj, out-proj) are interleaved as
"filler" work paced by a simple time model to keep the PE queue
always-ready: no PE stalls => the tensor engine stays at its top p-state
(2.4 GHz).  All matmul inputs are bf16 (halves DMA + SBUF vs f32).

PSUM layout (8 banks):
  sps0, sps1  [128,1024] 2 banks each -- score parities (kc%2), each
              holds both heads' scores for one 512-wide q window.
  u0a, u0b    [128, 512] 1 bank each -- head-0 AV accumulator, double-
              buffered across segments (hides the DVE normalize).
  u1          [128, 512] -- head-1 AV accumulator (normalized first).
  fb          [128, 512] -- filler accumulator (proj units).
"""

import os

import numpy as np
import ml_dtypes

import concourse.bass as bass
import concourse.bacc as bacc
import concourse.tile as tile
from concourse import mybir
from concourse.bass_utils import run_bass_kernel_spmd

N_CORES = 8
C = 1024           # embed dim
H = 16             # total heads
HD = 64            # head dim
HPC = 8            # heads per core
CG = HPC * HD      # 512: per-core q/k/v feature width
QW = 512           # q-window width per attention segment

F32 = mybir.dt.float32
BF16 = mybir.dt.bfloat16

# pacing model (ns) -- drives filler insertion only, not correctness.
# (A/B-tested on HW against calibrated-but-heavier pacing; this won.)
MM_NS, ACT_NS, DVE_NS, SEM_NS = 215.0, 1075.0, 710.0, 150.0
PAD_OK = False


def _body(tc, T, x_t, w_qk, b_qk, w_v, ones_bf, w_p, b_out, out_t):
    nc = tc.nc
    KC = C // 128           # 8: contraction chunks over C
    TC1 = T // 128          # key blocks (kc loop trip count)
    T4 = T // 512           # 512-wide token windows
    NP = HPC // 2           # 4 head pairs
    PCH = CG // 128         # 4 out-proj contraction chunks
    OCC = C // 128          # 8 out-channel chunks
    Exp = mybir.ActivationFunctionType.Exp
    Mult = mybir.AluOpType.mult

    with (
        tc.tile_pool(name="pers", bufs=1) as pers,
        tc.tile_pool(name="et", bufs=3) as ep,
        tc.tile_pool(name="rec", bufs=2) as recp,
        tc.tile_pool(name="osb", bufs=4) as outp,
        tc.tile_pool(name="ps", bufs=1, space="PSUM") as psp,
    ):
        # ---- persistent SBUF; DMA issue order = arrival order, so
        # interleave xt/wv per contraction chunk (the V prologue consumes
        # them kc-major while the rest still streams in).
        bqk_sb = pers.tile([128, 2 * CG // 128], F32, tag="bqk")
        nc.sync.dma_start(bqk_sb[:], b_qk[:])
        bout_sb = pers.tile([128, OCC], F32, tag="bout")
        nc.sync.dma_start(bout_sb[:], b_out[:])

        xt, wv = [], []
        for kc in range(KC):
            t = pers.tile([128, T], BF16, tag=f"xt{kc}", name=f"xt{kc}")
            nc.sync.dma_start(t[:], x_t[kc * 128:(kc + 1) * 128, :])
            xt.append(t)
            t = pers.tile([128, CG], BF16, tag=f"wv{kc}", name=f"wv{kc}")
            nc.sync.dma_start(t[:], w_v[kc * 128:(kc + 1) * 128, :])
            wv.append(t)
        wqk = {}
        for fc in range(2 * CG // 128):
            for kc in range(KC):
                t = pers.tile([128, 128], BF16, tag=f"wqk_{fc}_{kc}",
                              name=f"wqk_{fc}_{kc}")
                nc.sync.dma_start(
                    t[:], w_qk[kc * 128:(kc + 1) * 128,
                               fc * 128:(fc + 1) * 128])
                wqk[(fc, kc)] = t
        wp = []
        for f in range(PCH):
            t = pers.tile([128, C], BF16, tag=f"wp{f}", name=f"wp{f}")
            nc.sync.dma_start(t[:], w_p[f * 128:(f + 1) * 128, :])
            wp.append(t)

        # per-head blocks of 128 cols: [V_h (64) | ones (64)] so one M=128
        # AV matmul yields U^T rows 0:64 and the replicated softmax
        # denominator rows 64:128.
        v2 = [pers.tile([128, 2 * CG], BF16, tag=f"v2_{i}", name=f"v2_{i}")
              for i in range(TC1)]
        qkt = [pers.tile([128, T], BF16, tag=f"qkt{f}", name=f"qkt{f}")
               for f in range(2 * CG // 128)]
        yt = [pers.tile([128, T], BF16, tag=f"yt{p}", name=f"yt{p}")
              for p in range(NP)]

        # ---- pacing model state (drives filler insertion only) ----
        st = {"pe": 0.0, "act": 0.0, "dve": 0.0, "uid": 0}

        def mm():
            st["pe"] += MM_NS

        def dve():
            st["dve"] = max(st["dve"], st["pe"] + SEM_NS) + DVE_NS

        def v_drain(tokc, ps):
            v2v = v2[tokc].rearrange("p (h c) -> p h c", c=2 * HD)
            nc.vector.tensor_copy(
                v2v[:, :, 0:HD], ps.rearrange("p (h c) -> p h c", c=HD))
            dve()
            nc.sync.dma_start(
                v2v[:, :, HD:2 * HD],
                ones_bf.rearrange("p (h c) -> p h c", c=HD))

        # ---- V-projection prologue: kc-major in batches of 4 token
        # blocks so the PE starts as soon as xt[0]/wv[0] land; batch b and
        # b+1 use disjoint psum banks so b's drains have a whole batch of
        # slack.
        slotsA = ("sps0", "sps1")        # sliced in halves -> 4 slots
        slotsB = ("u0a", "u0b", "u1", "fb")
        for b0 in range(0, TC1, 4):
            if (b0 // 4) % 2 == 0:
                g0 = psp.tile([128, 1024], F32, tag=slotsA[0],
                              name=f"vpro{b0}_0")
                g1 = psp.tile([128, 1024], F32, tag=slotsA[1],
                              name=f"vpro{b0}_1")
                slots = [g0[:, 0:512], g0[:, 512:1024],
                         g1[:, 0:512], g1[:, 512:1024]]
            else:
                slots = [psp.tile([128, 512], F32, tag=tg,
                                  name=f"vpro{b0}_{tg}") for tg in slotsB]
            n = min(4, TC1 - b0)
            for kc in range(KC):
                for j in range(n):
                    nc.tensor.matmul(
                        slots[j], xt[kc][:, (b0 + j) * 128:(b0 + j + 1) * 128],
                        wv[kc][:], start=(kc == 0), stop=(kc == KC - 1))
                    mm()
            for j in range(n):
                v_drain(b0 + j, slots[j])

        # ---- filler units: (name, mm-emitters factory, drain) ----
        def qk_unit(fc, t4):
            def mms(ps):
                out = []
                for kc in range(KC):
                    def e(kc=kc, ps=ps):
                        nc.tensor.matmul(
                            ps[:], wqk[(fc, kc)][:],
                            xt[kc][:, t4 * 512:(t4 + 1) * 512],
                            start=(kc == 0), stop=(kc == KC - 1))
                        mm()
                    out.append(e)
                return out

            def drain(ps):
                nc.vector.tensor_scalar_add(
                    qkt[fc][:, t4 * 512:(t4 + 1) * 512], ps[:],
                    bqk_sb[:, fc:fc + 1])
                dve()
            return (f"qk_{fc}_{t4}", mms, drain)

        def o_unit(occ, t4):
            def mms(ps):
                out = []
                for f in range(PCH):
                    def e(f=f, ps=ps):
                        nc.tensor.matmul(
                            ps[:], wp[f][:, occ * 128:(occ + 1) * 128],
                            yt[f][:, t4 * 512:(t4 + 1) * 512],
                            start=(f == 0), stop=(f == PCH - 1))
                        mm()
                    out.append(e)
                return out

            def drain(ps):
                osb = outp.tile([128, 512], F32, tag="osb",
                                name=f"osb_{occ}_{t4}")
                nc.vector.tensor_scalar_add(osb[:], ps[:],
                                            bout_sb[:, occ:occ + 1])
                dve()
                nc.sync.dma_start(
                    out_t[occ * 128:(occ + 1) * 128,
                          t4 * 512:(t4 + 1) * 512], osb[:])
            return (f"o_{occ}_{t4}", mms, drain)

        # filler machinery: deadline-ordered queue; emit_filler() emits ONE
        # matmul (psum slot allocated on first touch, DVE drain right after
        # the last matmul).
        queue = []
        cur = {"mms": None, "i": 0, "drain": None, "ps": None, "name": None}
        done_units = set()

        def emit_filler(slot_tags=("fb",)):
            if cur["mms"] is None:
                if not queue:
                    return False
                name, mmf, drain = queue.pop(0)
                tag = slot_tags[st["uid"] % len(slot_tags)]
                ps = psp.tile([128, 512], F32, tag=tag, name=f"acc_{name}")
                st["uid"] += 1
                cur.update(mms=mmf(ps), i=0, drain=drain, ps=ps, name=name)
            cur["mms"][cur["i"]]()
            cur["i"] += 1
            if cur["i"] >= len(cur["mms"]):
                cur["drain"](cur["ps"])
                done_units.add(cur["name"])
                cur["mms"] = None
            return True

        def emit_pad():
            # only when no unit is mid-accumulation in fb
            if cur["mms"] is not None:
                return emit_filler(("fb",))
            ps = psp.tile([128, 512], F32, tag="fb",
                          name=f"pad{st['uid']}")
            st["uid"] += 1
            nc.tensor.matmul(ps[:], wqk[(0, 0)][:], xt[0][:, 0:512],
                             start=True, stop=True)
            mm()
            return True

        def force(name, slot_tags=("fb",)):
            while name not in done_units:
                assert emit_filler(slot_tags), f"filler underflow at {name}"

        # ---- rest of prologue: K0, Q(0,0) first (their DVE drains settle
        # while K1/K2 stream), then K1, K2 for the first window group.
        for t4 in range(T4):
            queue.append(qk_unit(NP + 0, t4))
        queue.append(qk_unit(0, 0))
        for p in (1, 2):
            for t4 in range(T4):
                queue.append(qk_unit(NP + p, t4))
        while emit_filler(("fb", "u0a", "u0b", "u1")):
            pass

        # ---- stream queue for window group 0 ----
        queue.extend([qk_unit(1, 0), qk_unit(2, 0)])
        for t4 in range(T4):
            queue.append(qk_unit(NP + 3, t4))
        queue.append(qk_unit(3, 0))

        # ---- attention segments: window-outer (qc-outer), pair inner, so
        # out-proj for window w unlocks after only 4 segments.
        seg_idx = 0
        u1_free = 0.0
        for w in range(T4):
            q0 = w * QW
            for p in range(NP):
                force(f"qk_{p}_{w}")
                qt, kt = qkt[p], qkt[NP + p]
                u0_tag = ("u0a", "u0b")[seg_idx % 2]
                u0_other = ("u0a", "u0b")[(seg_idx + 1) % 2]
                u0 = psp.tile([128, 512], F32, tag=u0_tag,
                              name=f"u0_{p}_{w}")
                u1t = psp.tile([128, 512], F32, tag="u1", name=f"u1_{p}_{w}")
                ups = (u0, u1t)
                ets = {}
                act_done = {}

                def emit_S(kc):
                    sgen = psp.tile([128, 1024], F32, tag=f"sps{kc % 2}",
                                    name=f"sps_{p}_{w}_{kc}")
                    for s in (0, 1):
                        po = s * 64
                        nc.tensor.matmul(
                            sgen[:, s * 512:(s + 1) * 512],
                            kt[po:po + 64, kc * 128:(kc + 1) * 128],
                            qt[po:po + 64, q0:q0 + QW],
                            start=True, stop=True)
                        mm()
                    et = ep.tile([128, 1024], BF16, tag="et",
                                 name=f"et_{p}_{w}_{kc}")
                    nc.scalar.activation(et[:], sgen[:], Exp, scale=0.125)
                    st["act"] = max(st["act"], st["pe"] + SEM_NS) + ACT_NS
                    act_done[kc] = st["act"]
                    ets[kc] = et

                def emit_A(kc):
                    et = ets.pop(kc)
                    for s in (0, 1):
                        nc.tensor.matmul(
                            ups[s][:],
                            v2[kc][:, (2 * p + s) * 128:
                                   (2 * p + s + 1) * 128],
                            et[:, s * 512:(s + 1) * 512],
                            start=(kc == 0), stop=(kc == TC1 - 1))
                        mm()

                # segment pipeline: S runs two kc ahead of A.  Interior
                # fillers may also use the idle u0 parity bank (their drain
                # finishes long before the next segment claims it).
                emit_S(0)
                emit_S(1)
                for kc in range(TC1):
                    slot_tags = (("fb", u0_other)
                                 if 2 <= kc < TC1 - 6 else ("fb",))
                    tgt = act_done[kc] + SEM_NS
                    if kc == 0:
                        tgt = max(tgt, u1_free + SEM_NS)
                    while st["pe"] < tgt - MM_NS:
                        if not emit_filler(slot_tags) and not (
                                PAD_OK and emit_pad()):
                            break
                    emit_A(kc)
                    if kc + 2 < TC1:
                        emit_S(kc + 2)

                # normalize: head 1 first (its accumulator is
                # single-buffered and blocks the next segment's A(0))
                for s in (1, 0):
                    po = s * 64
                    rec = recp.tile([64, QW], F32, tag="rec",
                                    name=f"rec_{p}_{w}_{s}")
                    nc.vector.reciprocal(rec[:], ups[s][64:128, :])
                    dve()
                    nc.vector.tensor_tensor(
                        yt[p][po:po + 64, q0:q0 + QW],
                        ups[s][0:64, :], rec[:], op=Mult)
                    if s == 1:
                        u1_free = st["dve"] + DVE_NS
                    dve()
                seg_idx += 1
            # group w done: queue next window's Q chunks, then the now-legal
            # out-proj units for window w
            if w + 1 < T4:
                for p in range(NP):
                    queue.append(qk_unit(p, w + 1))
            for occ in range(OCC):
                queue.append(o_unit(occ, w))

        # ---- epilogue: drain remaining fillers (leftover out-proj) ----
        while emit_filler(("fb", "u0a", "u0b", "u1")):
            pass


def build_nc(T=2048):
    FC = 2 * CG // 128
    OCC = C // 128
    nc = bacc.Bacc("TRN2", target_bir_lowering=False, debug=False,
                   num_devices=N_CORES)
    x_t = nc.dram_tensor("x_t", [C, T], BF16, kind="ExternalInput")
    w_qk = nc.dram_tensor("w_qk", [C, 2 * CG], BF16, kind="ExternalInput")
    b_qk = nc.dram_tensor("b_qk", [128, FC], F32, kind="ExternalInput")
    w_v = nc.dram_tensor("w_v", [C, CG], BF16, kind="ExternalInput")
    ones_bf = nc.dram_tensor("ones_bf", [128, CG], BF16, kind="ExternalInput")
    w_p = nc.dram_tensor("w_p", [CG, C], BF16, kind="ExternalInput")
    b_out = nc.dram_tensor("b_out", [128, OCC], F32, kind="ExternalInput")
    out_t = nc.dram_tensor("out_t", [C, T], F32, kind="ExternalOutput")
    with tile.TileContext(nc) as tc:
        _body(tc, T, x_t.ap(), w_qk.ap(), b_qk.ap(), w_v.ap(), ones_bf.ap(),
              w_p.ap(), b_out.ap(), out_t.ap())
    nc.compile()
    return nc


def shard_inputs(sequences, w_attn, b_attn, w_proj, b_proj):
    """Build the 8 per-core input maps. Core index = b*2 + g."""
    sequences = np.asarray(sequences, dtype=np.float32)
    w_attn = np.asarray(w_attn, dtype=np.float32)
    b_attn = np.asarray(b_attn, dtype=np.float32)
    w_proj = np.asarray(w_proj, dtype=np.float32)
    b_proj = np.asarray(b_proj, dtype=np.float32)
    B = sequences.shape[0]
    in_maps = []
    for b in range(B):
        for g in range(2):
            qs = slice(g * CG, (g + 1) * CG)
            ks = slice(C + g * CG, C + (g + 1) * CG)
            vs = slice(2 * C + g * CG, 2 * C + (g + 1) * CG)
            in_maps.append({
                "x_t": np.ascontiguousarray(sequences[b].T)
                    .astype(ml_dtypes.bfloat16),
                "w_qk": np.ascontiguousarray(
                    np.concatenate([w_attn[:, qs], w_attn[:, ks]], axis=1))
                    .astype(ml_dtypes.bfloat16),
                "b_qk": np.ascontiguousarray(
                    np.concatenate([b_attn[qs], b_attn[ks]])
                    .reshape(8, 128).T),
                "w_v": np.ascontiguousarray(w_attn[:, vs])
                    .astype(ml_dtypes.bfloat16),
                "ones_bf": np.ones((128, CG), ml_dtypes.bfloat16),
                "w_p": np.ascontiguousarray(w_proj[g * CG:(g + 1) * CG, :])
                    .astype(ml_dtypes.bfloat16),
                # softmax rows sum to 1, so the v-bias folds into the output
                # bias: y_g = attn@(x@w_v) @ w_p + (b_v@w_p [+ b_proj on g0])
                "b_out": np.ascontiguousarray(
                    (b_attn[vs] @ w_proj[g * CG:(g + 1) * CG, :]
                     + (b_proj if g == 0 else 0.0))
                    .astype(np.float32).reshape(8, 128).T),
            })
    return in_maps


def unshard_outputs(outs, B, T):
    """outs: list of 8 [C, T] partials, core index = b*2+g."""
    y = np.empty((B, T, C), np.float32)
    for b in range(B):
        y[b] = (outs[2 * b] + outs[2 * b + 1]).T
    return y


_NC_CACHE = {}


def kernel(sequences, w_attn, b_attn, w_proj, b_proj):
    sequences = np.asarray(sequences, dtype=np.float32)
    B, T, _ = sequences.shape
    in_maps = shard_inputs(sequences, w_attn, b_attn, w_proj, b_proj)
    if T not in _NC_CACHE:
        _NC_CACHE[T] = build_nc(T)
    nc = _NC_CACHE[T]
    res = run_bass_kernel_spmd(nc, in_maps, list(range(N_CORES)))
    outs = [res.results[i]["out_t"] for i in range(N_CORES)]
    return unshard_outputs(outs, B, T)


if __name__ == "__main__":
    rng = np.random.default_rng(0)
    B, T = 4, 2048
    seq = rng.standard_normal((B, T, C), dtype=np.float32)
    wa = rng.standard_normal((C, 3 * C), dtype=np.float32) / np.sqrt(C)
    ba = np.zeros(3 * C, np.float32)
    wp = rng.standard_normal((C, C), dtype=np.float32) / np.sqrt(C)
    bp = np.zeros(C, np.float32)
    y = kernel(seq, wa, ba, wp, bp)
    print(y.shape, y.dtype)


# revision 11
# speedup vs baseline: 1.1009x; 1.0745x over previous
"""Multi-head self-attention Trainium2 Bass kernel (v2: fused stream).

Problem: y = (softmax((x@Wq)(x@Wk)^T / sqrt(hd)) (x@Wv)) @ Wp + biases
with B=4, T=2048, C=1024, H=16, hd=64.

Sharding over 8 NeuronCores: (batch b in 0..3) x (head-group g in 0..1, 8
heads each).  Each core computes attention for its batch and head group
plus the partial output projection restricted to its head group's
features; the host sums the two head-group partials per batch.

v2 design (vs the chunked baseline): ONE fused PE instruction stream.
The attention inner loop (per key-block kc: 2 score matmuls -> one fused
two-head exp [128,1024] on ScalarE -> 2 AV matmuls) is ACT/PE balanced,
so projection matmuls (QK-proj, V-proj, out-proj) are interleaved as
"filler" work paced by a simple time model to keep the PE queue
always-ready: no PE stalls => the tensor engine stays at its top p-state
(2.4 GHz).  All matmul inputs are bf16 (halves DMA + SBUF vs f32).

PSUM layout (8 banks):
  sps0, sps1  [128,1024] 2 banks each -- score parities (kc%2), each
              holds both heads' scores for one 512-wide q window.
  u0a, u0b    [128, 512] 1 bank each -- head-0 AV accumulator, double-
              buffered across segments (hides the DVE normalize).
  u1          [128, 512] -- head-1 AV accumulator (normalized first).
  fb          [128, 512] -- filler accumulator (proj units).
"""

import os

import numpy as np
import ml_dtypes

import concourse.bass as bass
import concourse.bacc as bacc
import concourse.tile as tile
from concourse import mybir
from concourse.bass_utils import run_bass_kernel_spmd

N_CORES = 8
C = 1024           # embed dim
H = 16             # total heads
HD = 64            # head dim
HPC = 8            # heads per core
CG = HPC * HD      # 512: per-core q/k/v feature width
QW = 512           # q-window width per attention segment

F32 = mybir.dt.float32
BF16 = mybir.dt.bfloat16

# pacing model (ns) -- drives filler insertion only, not correctness.
# (A/B-tested on HW against calibrated-but-heavier pacing; this won.)
# (A/B-tested on HW: heavier DVE/boundary pacing and pad-matmul variants
# both lost to this lighter schedule.)
MM_NS, ACT_NS, DVE_NS, SEM_NS = 215.0, 1075.0, 710.0, 150.0
PAD_OK = False


def _body(tc, T, x_t, w_qk, b_qk, w_v, ones_bf, w_p, b_out, out_t):
    nc = tc.nc
    KC = C // 128           # 8: contraction chunks over C
    TC1 = T // 128          # key blocks (kc loop trip count)
    T4 = T // 512           # 512-wide token windows
    NP = HPC // 2           # 4 head pairs
    PCH = CG // 128         # 4 out-proj contraction chunks
    OCC = C // 128          # 8 out-channel chunks
    Exp = mybir.ActivationFunctionType.Exp
    Mult = mybir.AluOpType.mult

    with (
        tc.tile_pool(name="pers", bufs=1) as pers,
        tc.tile_pool(name="et", bufs=4) as ep,
        tc.tile_pool(name="rec", bufs=2) as recp,
        tc.tile_pool(name="osb", bufs=6) as outp,
        tc.tile_pool(name="ps", bufs=1, space="PSUM") as psp,
    ):
        # ---- persistent SBUF; DMA issue order = arrival order, so
        # interleave xt/wv per contraction chunk (the V prologue consumes
        # them kc-major while the rest still streams in).
        bqk_sb = pers.tile([128, 2 * CG // 128], F32, tag="bqk")
        nc.sync.dma_start(bqk_sb[:], b_qk[:])
        bout_sb = pers.tile([128, OCC], F32, tag="bout")
        nc.sync.dma_start(bout_sb[:], b_out[:])

        xt, wv = [], []
        for kc in range(KC):
            t = pers.tile([128, T], BF16, tag=f"xt{kc}", name=f"xt{kc}")
            nc.sync.dma_start(t[:], x_t[kc * 128:(kc + 1) * 128, :])
            xt.append(t)
            t = pers.tile([128, CG], BF16, tag=f"wv{kc}", name=f"wv{kc}")
            nc.sync.dma_start(t[:], w_v[kc * 128:(kc + 1) * 128, :])
            wv.append(t)
        wqk = {}
        for fc in range(2 * CG // 128):
            for kc in range(KC):
                t = pers.tile([128, 128], BF16, tag=f"wqk_{fc}_{kc}",
                              name=f"wqk_{fc}_{kc}")
                nc.sync.dma_start(
                    t[:], w_qk[kc * 128:(kc + 1) * 128,
                               fc * 128:(fc + 1) * 128])
                wqk[(fc, kc)] = t
        wp = []
        for f in range(PCH):
            t = pers.tile([128, C], BF16, tag=f"wp{f}", name=f"wp{f}")
            nc.sync.dma_start(t[:], w_p[f * 128:(f + 1) * 128, :])
            wp.append(t)

        # per-head blocks of 128 cols: [V_h (64) | ones (64)] so one M=128
        # AV matmul yields U^T rows 0:64 and the replicated softmax
        # denominator rows 64:128.
        v2 = [pers.tile([128, 2 * CG], BF16, tag=f"v2_{i}", name=f"v2_{i}")
              for i in range(TC1)]
        qkt = [pers.tile([128, T], BF16, tag=f"qkt{f}", name=f"qkt{f}")
               for f in range(2 * CG // 128)]
        yt = [pers.tile([128, T], BF16, tag=f"yt{p}", name=f"yt{p}")
              for p in range(NP)]

        # ---- pacing model state (drives filler insertion only) ----
        st = {"pe": 0.0, "act": 0.0, "dve": 0.0, "uid": 0}

        def mm():
            st["pe"] += MM_NS

        def dve():
            st["dve"] = max(st["dve"], st["pe"] + SEM_NS) + DVE_NS

        def v_drain(tokc, ps):
            v2v = v2[tokc].rearrange("p (h c) -> p h c", c=2 * HD)
            nc.vector.tensor_copy(
                v2v[:, :, 0:HD], ps.rearrange("p (h c) -> p h c", c=HD))
            dve()
            nc.sync.dma_start(
                v2v[:, :, HD:2 * HD],
                ones_bf.rearrange("p (h c) -> p h c", c=HD))

        # ---- V-projection prologue: kc-major in batches of 4 token
        # blocks so the PE starts as soon as xt[0]/wv[0] land; batch b and
        # b+1 use disjoint psum banks so b's drains have a whole batch of
        # slack.
        slotsA = ("sps0", "sps1")        # sliced in halves -> 4 slots
        slotsB = ("u0a", "u0b", "u1", "fb")
        for b0 in range(0, TC1, 4):
            if (b0 // 4) % 2 == 0:
                g0 = psp.tile([128, 1024], F32, tag=slotsA[0],
                              name=f"vpro{b0}_0")
                g1 = psp.tile([128, 1024], F32, tag=slotsA[1],
                              name=f"vpro{b0}_1")
                slots = [g0[:, 0:512], g0[:, 512:1024],
                         g1[:, 0:512], g1[:, 512:1024]]
            else:
                slots = [psp.tile([128, 512], F32, tag=tg,
                                  name=f"vpro{b0}_{tg}") for tg in slotsB]
            n = min(4, TC1 - b0)
            for kc in range(KC):
                for j in range(n):
                    nc.tensor.matmul(
                        slots[j], xt[kc][:, (b0 + j) * 128:(b0 + j + 1) * 128],
                        wv[kc][:], start=(kc == 0), stop=(kc == KC - 1))
                    mm()
            for j in range(n):
                v_drain(b0 + j, slots[j])

        # ---- filler units: (name, mm-emitters factory, drain) ----
        def qk_unit(fc, t4):
            def mms(ps):
                out = []
                for kc in range(KC):
                    def e(kc=kc, ps=ps):
                        nc.tensor.matmul(
                            ps[:], wqk[(fc, kc)][:],
                            xt[kc][:, t4 * 512:(t4 + 1) * 512],
                            start=(kc == 0), stop=(kc == KC - 1))
                        mm()
                    out.append(e)
                return out

            def drain(ps):
                nc.vector.tensor_scalar_add(
                    qkt[fc][:, t4 * 512:(t4 + 1) * 512], ps[:],
                    bqk_sb[:, fc:fc + 1])
                dve()
            return (f"qk_{fc}_{t4}", mms, drain)

        def o_unit(occ, t4):
            def mms(ps):
                out = []
                for f in range(PCH):
                    def e(f=f, ps=ps):
                        nc.tensor.matmul(
                            ps[:], wp[f][:, occ * 128:(occ + 1) * 128],
                            yt[f][:, t4 * 512:(t4 + 1) * 512],
                            start=(f == 0), stop=(f == PCH - 1))
                        mm()
                    out.append(e)
                return out

            def drain(ps):
                osb = outp.tile([128, 512], F32, tag="osb",
                                name=f"osb_{occ}_{t4}")
                nc.vector.tensor_scalar_add(osb[:], ps[:],
                                            bout_sb[:, occ:occ + 1])
                dve()
                nc.sync.dma_start(
                    out_t[occ * 128:(occ + 1) * 128,
                          t4 * 512:(t4 + 1) * 512], osb[:])
            return (f"o_{occ}_{t4}", mms, drain)

        # filler machinery: deadline-ordered queue; emit_filler() emits ONE
        # matmul (psum slot allocated on first touch, DVE drain right after
        # the last matmul).
        queue = []
        cur = {"mms": None, "i": 0, "drain": None, "ps": None, "name": None}
        done_units = set()

        def emit_filler(slot_tags=("fb",)):
            if cur["mms"] is None:
                if not queue:
                    return False
                name, mmf, drain = queue.pop(0)
                tag = slot_tags[st["uid"] % len(slot_tags)]
                ps = psp.tile([128, 512], F32, tag=tag, name=f"acc_{name}")
                st["uid"] += 1
                cur.update(mms=mmf(ps), i=0, drain=drain, ps=ps, name=name)
            cur["mms"][cur["i"]]()
            cur["i"] += 1
            if cur["i"] >= len(cur["mms"]):
                cur["drain"](cur["ps"])
                done_units.add(cur["name"])
                cur["mms"] = None
            return True

        def emit_pad():
            # only when no unit is mid-accumulation in fb
            if cur["mms"] is not None:
                return emit_filler(("fb",))
            ps = psp.tile([128, 512], F32, tag="fb",
                          name=f"pad{st['uid']}")
            st["uid"] += 1
            nc.tensor.matmul(ps[:], wqk[(0, 0)][:], xt[0][:, 0:512],
                             start=True, stop=True)
            mm()
            return True

        def force(name, slot_tags=("fb",)):
            while name not in done_units:
                assert emit_filler(slot_tags), f"filler underflow at {name}"

        # ---- rest of prologue: K0, Q(0,0) first (their DVE drains settle
        # while K1/K2 stream), then K1, K2 for the first window group.
        for t4 in range(T4):
            queue.append(qk_unit(NP + 0, t4))
        queue.append(qk_unit(0, 0))
        for p in (1, 2):
            for t4 in range(T4):
                queue.append(qk_unit(NP + p, t4))
        while emit_filler(("fb", "u0a", "u0b", "u1")):
            pass

        # ---- stream queue for window group 0 ----
        queue.extend([qk_unit(1, 0), qk_unit(2, 0)])
        for t4 in range(T4):
            queue.append(qk_unit(NP + 3, t4))
        queue.append(qk_unit(3, 0))

        # ---- attention segments: window-outer (qc-outer), pair inner, so
        # out-proj for window w unlocks after only 4 segments.
        seg_idx = 0
        u1_free = 0.0
        for w in range(T4):
            q0 = w * QW
            for p in range(NP):
                force(f"qk_{p}_{w}")
                qt, kt = qkt[p], qkt[NP + p]
                u0_tag = ("u0a", "u0b")[seg_idx % 2]
                u0_other = ("u0a", "u0b")[(seg_idx + 1) % 2]
                u0 = psp.tile([128, 512], F32, tag=u0_tag,
                              name=f"u0_{p}_{w}")
                u1t = psp.tile([128, 512], F32, tag="u1", name=f"u1_{p}_{w}")
                ups = (u0, u1t)
                ets = {}
                act_done = {}

                def emit_S(kc):
                    sgen = psp.tile([128, 1024], F32, tag=f"sps{kc % 2}",
                                    name=f"sps_{p}_{w}_{kc}")
                    for s in (0, 1):
                        po = s * 64
                        nc.tensor.matmul(
                            sgen[:, s * 512:(s + 1) * 512],
                            kt[po:po + 64, kc * 128:(kc + 1) * 128],
                            qt[po:po + 64, q0:q0 + QW],
                            start=True, stop=True)
                        mm()
                    et = ep.tile([128, 1024], BF16, tag="et",
                                 name=f"et_{p}_{w}_{kc}")
                    nc.scalar.activation(et[:], sgen[:], Exp, scale=0.125)
                    st["act"] = max(st["act"], st["pe"] + SEM_NS) + ACT_NS
                    act_done[kc] = st["act"]
                    ets[kc] = et

                def emit_A(kc):
                    et = ets.pop(kc)
                    for s in (0, 1):
                        nc.tensor.matmul(
                            ups[s][:],
                            v2[kc][:, (2 * p + s) * 128:
                                   (2 * p + s + 1) * 128],
                            et[:, s * 512:(s + 1) * 512],
                            start=(kc == 0), stop=(kc == TC1 - 1))
                        mm()

                # segment pipeline: S runs two kc ahead of A.  Interior
                # fillers may also use the idle u0 parity bank (their drain
                # finishes long before the next segment claims it).
                emit_S(0)
                emit_S(1)
                for kc in range(TC1):
                    slot_tags = (("fb", u0_other)
                                 if 2 <= kc < TC1 - 6 else ("fb",))
                    tgt = act_done[kc] + SEM_NS
                    if kc == 0:
                        tgt = max(tgt, u1_free + SEM_NS)
                    while st["pe"] < tgt - MM_NS:
                        if not emit_filler(slot_tags) and not (
                                PAD_OK and emit_pad()):
                            break
                    emit_A(kc)
                    if kc + 2 < TC1:
                        emit_S(kc + 2)

                # normalize: head 1 first (its accumulator is
                # single-buffered and blocks the next segment's A(0))
                for s in (1, 0):
                    po = s * 64
                    rec = recp.tile([64, QW], F32, tag="rec",
                                    name=f"rec_{p}_{w}_{s}")
                    nc.vector.reciprocal(rec[:], ups[s][64:128, :])
                    dve()
                    nc.vector.tensor_tensor(
                        yt[p][po:po + 64, q0:q0 + QW],
                        ups[s][0:64, :], rec[:], op=Mult)
                    if s == 1:
                        u1_free = st["dve"] + DVE_NS
                    dve()
                seg_idx += 1
            # group w done: queue next window's Q chunks, then the now-legal
            # out-proj units for window w
            if w + 1 < T4:
                for p in range(NP):
                    queue.append(qk_unit(p, w + 1))
            for occ in range(OCC):
                queue.append(o_unit(occ, w))

        # ---- epilogue: drain remaining fillers (leftover out-proj) ----
        while emit_filler(("fb", "u0a", "u0b", "u1")):
            pass


def build_nc(T=2048):
    FC = 2 * CG // 128
    OCC = C // 128
    nc = bacc.Bacc("TRN2", target_bir_lowering=False, debug=False,
                   num_devices=N_CORES)
    x_t = nc.dram_tensor("x_t", [C, T], BF16, kind="ExternalInput")
    w_qk = nc.dram_tensor("w_qk", [C, 2 * CG], BF16, kind="ExternalInput")
    b_qk = nc.dram_tensor("b_qk", [128, FC], F32, kind="ExternalInput")
    w_v = nc.dram_tensor("w_v", [C, CG], BF16, kind="ExternalInput")
    ones_bf = nc.dram_tensor("ones_bf", [128, CG], BF16, kind="ExternalInput")
    w_p = nc.dram_tensor("w_p", [CG, C], BF16, kind="ExternalInput")
    b_out = nc.dram_tensor("b_out", [128, OCC], F32, kind="ExternalInput")
    out_t = nc.dram_tensor("out_t", [C, T], F32, kind="ExternalOutput")
    with tile.TileContext(nc) as tc:
        _body(tc, T, x_t.ap(), w_qk.ap(), b_qk.ap(), w_v.ap(), ones_bf.ap(),
              w_p.ap(), b_out.ap(), out_t.ap())
    nc.compile()
    return nc


def shard_inputs(sequences, w_attn, b_attn, w_proj, b_proj):
    """Build the 8 per-core input maps. Core index = b*2 + g."""
    sequences = np.asarray(sequences, dtype=np.float32)
    w_attn = np.asarray(w_attn, dtype=np.float32)
    b_attn = np.asarray(b_attn, dtype=np.float32)
    w_proj = np.asarray(w_proj, dtype=np.float32)
    b_proj = np.asarray(b_proj, dtype=np.float32)
    B = sequences.shape[0]
    in_maps = []
    for b in range(B):
        for g in range(2):
            qs = slice(g * CG, (g + 1) * CG)
            ks = slice(C + g * CG, C + (g + 1) * CG)
            vs = slice(2 * C + g * CG, 2 * C + (g + 1) * CG)
            in_maps.append({
                "x_t": np.ascontiguousarray(sequences[b].T)
                    .astype(ml_dtypes.bfloat16),
                "w_qk": np.ascontiguousarray(
                    np.concatenate([w_attn[:, qs], w_attn[:, ks]], axis=1))
                    .astype(ml_dtypes.bfloat16),
                "b_qk": np.ascontiguousarray(
                    np.concatenate([b_attn[qs], b_attn[ks]])
                    .reshape(8, 128).T),
                "w_v": np.ascontiguousarray(w_attn[:, vs])
                    .astype(ml_dtypes.bfloat16),
                "ones_bf": np.ones((128, CG), ml_dtypes.bfloat16),
                "w_p": np.ascontiguousarray(w_proj[g * CG:(g + 1) * CG, :])
                    .astype(ml_dtypes.bfloat16),
                # softmax rows sum to 1, so the v-bias folds into the output
                # bias: y_g = attn@(x@w_v) @ w_p + (b_v@w_p [+ b_proj on g0])
                "b_out": np.ascontiguousarray(
                    (b_attn[vs] @ w_proj[g * CG:(g + 1) * CG, :]
                     + (b_proj if g == 0 else 0.0))
                    .astype(np.float32).reshape(8, 128).T),
            })
    return in_maps


def unshard_outputs(outs, B, T):
    """outs: list of 8 [C, T] partials, core index = b*2+g."""
    y = np.empty((B, T, C), np.float32)
    for b in range(B):
        y[b] = (outs[2 * b] + outs[2 * b + 1]).T
    return y


_NC_CACHE = {}


def kernel(sequences, w_attn, b_attn, w_proj, b_proj):
    sequences = np.asarray(sequences, dtype=np.float32)
    B, T, _ = sequences.shape
    in_maps = shard_inputs(sequences, w_attn, b_attn, w_proj, b_proj)
    if T not in _NC_CACHE:
        _NC_CACHE[T] = build_nc(T)
    nc = _NC_CACHE[T]
    res = run_bass_kernel_spmd(nc, in_maps, list(range(N_CORES)))
    outs = [res.results[i]["out_t"] for i in range(N_CORES)]
    return unshard_outputs(outs, B, T)


if __name__ == "__main__":
    rng = np.random.default_rng(0)
    B, T = 4, 2048
    seq = rng.standard_normal((B, T, C), dtype=np.float32)
    wa = rng.standard_normal((C, 3 * C), dtype=np.float32) / np.sqrt(C)
    ba = np.zeros(3 * C, np.float32)
    wp = rng.standard_normal((C, C), dtype=np.float32) / np.sqrt(C)
    bp = np.zeros(C, np.float32)
    y = kernel(seq, wa, ba, wp, bp)
    print(y.shape, y.dtype)


# revision 14
# speedup vs baseline: 1.1164x; 1.0141x over previous
"""Multi-head self-attention Trainium2 Bass kernel (v2: fused stream).

Problem: y = (softmax((x@Wq)(x@Wk)^T / sqrt(hd)) (x@Wv)) @ Wp + biases
with B=4, T=2048, C=1024, H=16, hd=64.

Sharding over 8 NeuronCores: (batch b in 0..3) x (head-group g in 0..1, 8
heads each).  Each core computes attention for its batch and head group
plus the partial output projection restricted to its head group's
features; the host sums the two head-group partials per batch.

v2 design (vs the chunked baseline): ONE fused PE instruction stream.
The attention inner loop (per key-block kc: 2 score matmuls -> one fused
two-head exp [128,1024] on ScalarE -> 2 AV matmuls) is ACT/PE balanced,
so projection matmuls (QK-proj, V-proj, out-proj) are interleaved as
"filler" work paced by a simple time model to keep the PE queue
always-ready: no PE stalls => the tensor engine stays at its top p-state
(2.4 GHz).  All matmul inputs are bf16 (halves DMA + SBUF vs f32).

PSUM layout (8 banks):
  sps0, sps1  [128,1024] 2 banks each -- score parities (kc%2), each
              holds both heads' scores for one 512-wide q window.
  u0a, u0b    [128, 512] 1 bank each -- head-0 AV accumulator, double-
              buffered across segments (hides the DVE normalize).
  u1          [128, 512] -- head-1 AV accumulator (normalized first).
  fb          [128, 512] -- filler accumulator (proj units).
"""

import os

import numpy as np
import ml_dtypes

import concourse.bass as bass
import concourse.bacc as bacc
import concourse.tile as tile
from concourse import mybir
from concourse.bass_utils import run_bass_kernel_spmd

N_CORES = 8
C = 1024           # embed dim
H = 16             # total heads
HD = 64            # head dim
HPC = 8            # heads per core
CG = HPC * HD      # 512: per-core q/k/v feature width
QW = 512           # q-window width per attention segment

F32 = mybir.dt.float32
BF16 = mybir.dt.bfloat16

# pacing model (ns) -- drives filler insertion only, not correctness.
# (A/B-tested on HW against calibrated-but-heavier pacing; this won.)
# (A/B-tested on HW: heavier DVE/boundary pacing and pad-matmul variants
# both lost to this lighter schedule.)
MM_NS, ACT_NS, DVE_NS, SEM_NS = 215.0, 1075.0, 710.0, 150.0
PAD_OK = False


def _body(tc, T, x_t, w_qk, b_qk, w_v, ones_bf, w_p, b_out, out_t):
    nc = tc.nc
    KC = C // 128           # 8: contraction chunks over C
    TC1 = T // 128          # key blocks (kc loop trip count)
    T4 = T // 512           # 512-wide token windows
    NP = HPC // 2           # 4 head pairs
    PCH = CG // 128         # 4 out-proj contraction chunks
    OCC = C // 128          # 8 out-channel chunks
    Exp = mybir.ActivationFunctionType.Exp
    Mult = mybir.AluOpType.mult

    with (
        tc.tile_pool(name="pers", bufs=1) as pers,
        tc.tile_pool(name="et", bufs=4) as ep,
        tc.tile_pool(name="rec", bufs=2) as recp,
        tc.tile_pool(name="osb", bufs=6) as outp,
        tc.tile_pool(name="ps", bufs=1, space="PSUM") as psp,
    ):
        # ---- persistent SBUF; DMA issue order = arrival order, so
        # interleave xt/wv per contraction chunk (the V prologue consumes
        # them kc-major while the rest still streams in).
        bqk_sb = pers.tile([128, 2 * CG // 128], F32, tag="bqk")
        nc.sync.dma_start(bqk_sb[:], b_qk[:])
        bout_sb = pers.tile([128, OCC], F32, tag="bout")
        nc.sync.dma_start(bout_sb[:], b_out[:])

        xt, wv = [], []
        W0 = min(512, T)
        for kc in range(KC):
            t = pers.tile([128, T], BF16, tag=f"xt{kc}", name=f"xt{kc}")
            nc.sync.dma_start(t[:, 0:W0], x_t[kc * 128:(kc + 1) * 128, 0:W0])
            xt.append(t)
            t = pers.tile([128, CG], BF16, tag=f"wv{kc}", name=f"wv{kc}")
            nc.sync.dma_start(t[:], w_v[kc * 128:(kc + 1) * 128, :])
            wv.append(t)
        # one [128, 2CG] block DMA per contraction chunk (8 DMAs instead
        # of 64 -- HWDGE descriptor generation is ~0.6us per DMA), second
        # xt wave interleaved so the V prologue stays fed.
        wqk_blk = []
        for kc in range(KC):
            if T > W0:
                nc.sync.dma_start(xt[kc][:, W0:T],
                                  x_t[kc * 128:(kc + 1) * 128, W0:T])
            t = pers.tile([128, 2 * CG], BF16, tag=f"wqkb{kc}",
                          name=f"wqkb{kc}")
            nc.sync.dma_start(t[:], w_qk[kc * 128:(kc + 1) * 128, :])
            wqk_blk.append(t)
        wqk = {(fc, kc): wqk_blk[kc][:, fc * 128:(fc + 1) * 128]
               for fc in range(2 * CG // 128) for kc in range(KC)}
        wp = []
        for f in range(PCH):
            t = pers.tile([128, C], BF16, tag=f"wp{f}", name=f"wp{f}")
            nc.sync.dma_start(t[:], w_p[f * 128:(f + 1) * 128, :])
            wp.append(t)

        # per-head blocks of 128 cols: [V_h (64) | ones (64)] so one M=128
        # AV matmul yields U^T rows 0:64 and the replicated softmax
        # denominator rows 64:128.
        v2 = [pers.tile([128, 2 * CG], BF16, tag=f"v2_{i}", name=f"v2_{i}")
              for i in range(TC1)]
        qkt = [pers.tile([128, T], BF16, tag=f"qkt{f}", name=f"qkt{f}")
               for f in range(2 * CG // 128)]
        yt = [pers.tile([128, T], BF16, tag=f"yt{p}", name=f"yt{p}")
              for p in range(NP)]

        # ---- pacing model state (drives filler insertion only) ----
        st = {"pe": 0.0, "act": 0.0, "dve": 0.0, "uid": 0}

        def mm():
            st["pe"] += MM_NS

        def dve():
            st["dve"] = max(st["dve"], st["pe"] + SEM_NS) + DVE_NS

        def v_drain(tokc, ps):
            v2v = v2[tokc].rearrange("p (h c) -> p h c", c=2 * HD)
            nc.vector.tensor_copy(
                v2v[:, :, 0:HD], ps.rearrange("p (h c) -> p h c", c=HD))
            dve()
            nc.sync.dma_start(
                v2v[:, :, HD:2 * HD],
                ones_bf.rearrange("p (h c) -> p h c", c=HD))

        # ---- V-projection prologue: kc-major in batches of 4 token
        # blocks so the PE starts as soon as xt[0]/wv[0] land; batch b and
        # b+1 use disjoint psum banks so b's drains have a whole batch of
        # slack.
        slotsA = ("sps0", "sps1")        # sliced in halves -> 4 slots
        slotsB = ("u0a", "u0b", "u1", "fb")
        for b0 in range(0, TC1, 4):
            if (b0 // 4) % 2 == 0:
                g0 = psp.tile([128, 1024], F32, tag=slotsA[0],
                              name=f"vpro{b0}_0")
                g1 = psp.tile([128, 1024], F32, tag=slotsA[1],
                              name=f"vpro{b0}_1")
                slots = [g0[:, 0:512], g0[:, 512:1024],
                         g1[:, 0:512], g1[:, 512:1024]]
            else:
                slots = [psp.tile([128, 512], F32, tag=tg,
                                  name=f"vpro{b0}_{tg}") for tg in slotsB]
            n = min(4, TC1 - b0)
            for kc in range(KC):
                for j in range(n):
                    nc.tensor.matmul(
                        slots[j], xt[kc][:, (b0 + j) * 128:(b0 + j + 1) * 128],
                        wv[kc][:], start=(kc == 0), stop=(kc == KC - 1))
                    mm()
            for j in range(n):
                v_drain(b0 + j, slots[j])

        # ---- filler units: (name, mm-emitters factory, drain) ----
        def qk_unit(fc, t4):
            def mms(ps):
                out = []
                for kc in range(KC):
                    def e(kc=kc, ps=ps):
                        nc.tensor.matmul(
                            ps[:], wqk[(fc, kc)][:],
                            xt[kc][:, t4 * 512:(t4 + 1) * 512],
                            start=(kc == 0), stop=(kc == KC - 1))
                        mm()
                    out.append(e)
                return out

            def drain(ps):
                nc.vector.tensor_scalar_add(
                    qkt[fc][:, t4 * 512:(t4 + 1) * 512], ps[:],
                    bqk_sb[:, fc:fc + 1])
                dve()
            return (f"qk_{fc}_{t4}", mms, drain)

        def o_unit(occ, t4):
            def mms(ps):
                out = []
                for f in range(PCH):
                    def e(f=f, ps=ps):
                        nc.tensor.matmul(
                            ps[:], wp[f][:, occ * 128:(occ + 1) * 128],
                            yt[f][:, t4 * 512:(t4 + 1) * 512],
                            start=(f == 0), stop=(f == PCH - 1))
                        mm()
                    out.append(e)
                return out

            def drain(ps):
                osb = outp.tile([128, 512], F32, tag="osb",
                                name=f"osb_{occ}_{t4}")
                nc.vector.tensor_scalar_add(osb[:], ps[:],
                                            bout_sb[:, occ:occ + 1])
                dve()
                nc.sync.dma_start(
                    out_t[occ * 128:(occ + 1) * 128,
                          t4 * 512:(t4 + 1) * 512], osb[:])
            return (f"o_{occ}_{t4}", mms, drain)

        # filler machinery: deadline-ordered queue; emit_filler() emits ONE
        # matmul (psum slot allocated on first touch, DVE drain right after
        # the last matmul).
        queue = []
        cur = {"mms": None, "i": 0, "drain": None, "ps": None, "name": None}
        done_units = set()

        def emit_filler(slot_tags=("fb",)):
            if cur["mms"] is None:
                if not queue:
                    return False
                name, mmf, drain = queue.pop(0)
                tag = slot_tags[st["uid"] % len(slot_tags)]
                ps = psp.tile([128, 512], F32, tag=tag, name=f"acc_{name}")
                st["uid"] += 1
                cur.update(mms=mmf(ps), i=0, drain=drain, ps=ps, name=name)
            cur["mms"][cur["i"]]()
            cur["i"] += 1
            if cur["i"] >= len(cur["mms"]):
                cur["drain"](cur["ps"])
                done_units.add(cur["name"])
                cur["mms"] = None
            return True

        def emit_pad():
            # only when no unit is mid-accumulation in fb
            if cur["mms"] is not None:
                return emit_filler(("fb",))
            ps = psp.tile([128, 512], F32, tag="fb",
                          name=f"pad{st['uid']}")
            st["uid"] += 1
            nc.tensor.matmul(ps[:], wqk[(0, 0)][:], xt[0][:, 0:512],
                             start=True, stop=True)
            mm()
            return True

        def force(name, slot_tags=("fb",)):
            while name not in done_units:
                assert emit_filler(slot_tags), f"filler underflow at {name}"

        # ---- rest of prologue: K0, Q(0,0) first (their DVE drains settle
        # while K1/K2 stream), then K1, K2 for the first window group.
        for t4 in range(T4):
            queue.append(qk_unit(NP + 0, t4))
        queue.append(qk_unit(0, 0))
        for p in (1, 2):
            for t4 in range(T4):
                queue.append(qk_unit(NP + p, t4))
        while emit_filler(("fb", "u0a", "u0b", "u1")):
            pass

        # ---- stream queue for window group 0 ----
        queue.extend([qk_unit(1, 0), qk_unit(2, 0)])
        for t4 in range(T4):
            queue.append(qk_unit(NP + 3, t4))
        queue.append(qk_unit(3, 0))

        # ---- attention segments: window-outer (qc-outer), pair inner, so
        # out-proj for window w unlocks after only 4 segments.
        seg_idx = 0
        u1_free = 0.0
        for w in range(T4):
            q0 = w * QW
            for p in range(NP):
                if p == NP - 1 and w + 1 < T4:
                    # deadline order: ahead of any leftover out-proj units
                    queue[0:0] = [qk_unit(pn, w + 1) for pn in range(NP)]
                force(f"qk_{p}_{w}")
                qt, kt = qkt[p], qkt[NP + p]
                u0_tag = ("u0a", "u0b")[seg_idx % 2]
                u0_other = ("u0a", "u0b")[(seg_idx + 1) % 2]
                u0 = psp.tile([128, 512], F32, tag=u0_tag,
                              name=f"u0_{p}_{w}")
                u1t = psp.tile([128, 512], F32, tag="u1", name=f"u1_{p}_{w}")
                ups = (u0, u1t)
                ets = {}
                act_done = {}

                def emit_S(kc):
                    sgen = psp.tile([128, 1024], F32, tag=f"sps{kc % 2}",
                                    name=f"sps_{p}_{w}_{kc}")
                    for s in (0, 1):
                        po = s * 64
                        nc.tensor.matmul(
                            sgen[:, s * 512:(s + 1) * 512],
                            kt[po:po + 64, kc * 128:(kc + 1) * 128],
                            qt[po:po + 64, q0:q0 + QW],
                            start=True, stop=True)
                        mm()
                    et = ep.tile([128, 1024], BF16, tag="et",
                                 name=f"et_{p}_{w}_{kc}")
                    nc.scalar.activation(et[:], sgen[:], Exp, scale=0.125)
                    st["act"] = max(st["act"], st["pe"] + SEM_NS) + ACT_NS
                    act_done[kc] = st["act"]
                    ets[kc] = et

                def emit_A(kc):
                    et = ets.pop(kc)
                    for s in (0, 1):
                        nc.tensor.matmul(
                            ups[s][:],
                            v2[kc][:, (2 * p + s) * 128:
                                   (2 * p + s + 1) * 128],
                            et[:, s * 512:(s + 1) * 512],
                            start=(kc == 0), stop=(kc == TC1 - 1))
                        mm()

                # segment pipeline: S runs two kc ahead of A.  Interior
                # fillers may also use the idle u0 parity bank (their drain
                # finishes long before the next segment claims it).
                emit_S(0)
                emit_S(1)
                for kc in range(TC1):
                    slot_tags = (("fb", u0_other)
                                 if 2 <= kc < TC1 - 6 else ("fb",))
                    tgt = act_done[kc] + SEM_NS
                    if kc == 0:
                        tgt = max(tgt, u1_free + SEM_NS)
                    while st["pe"] < tgt - MM_NS:
                        if not emit_filler(slot_tags) and not (
                                PAD_OK and emit_pad()):
                            break
                    emit_A(kc)
                    if kc + 2 < TC1:
                        emit_S(kc + 2)

                # normalize: head 1 first (its accumulator is
                # single-buffered and blocks the next segment's A(0))
                for s in (1, 0):
                    po = s * 64
                    rec = recp.tile([64, QW], F32, tag="rec",
                                    name=f"rec_{p}_{w}_{s}")
                    nc.vector.reciprocal(rec[:], ups[s][64:128, :])
                    dve()
                    nc.vector.tensor_tensor(
                        yt[p][po:po + 64, q0:q0 + QW],
                        ups[s][0:64, :], rec[:], op=Mult)
                    if s == 1:
                        u1_free = st["dve"] + DVE_NS
                    dve()
                seg_idx += 1
            # group w done: release the now-legal out-proj units
            for occ in range(OCC):
                queue.append(o_unit(occ, w))

        # ---- epilogue: drain remaining fillers (leftover out-proj) ----
        while emit_filler(("fb", "u0a", "u0b", "u1")):
            pass


def build_nc(T=2048):
    FC = 2 * CG // 128
    OCC = C // 128
    nc = bacc.Bacc("TRN2", target_bir_lowering=False, debug=False,
                   num_devices=N_CORES)
    x_t = nc.dram_tensor("x_t", [C, T], BF16, kind="ExternalInput")
    w_qk = nc.dram_tensor("w_qk", [C, 2 * CG], BF16, kind="ExternalInput")
    b_qk = nc.dram_tensor("b_qk", [128, FC], F32, kind="ExternalInput")
    w_v = nc.dram_tensor("w_v", [C, CG], BF16, kind="ExternalInput")
    ones_bf = nc.dram_tensor("ones_bf", [128, CG], BF16, kind="ExternalInput")
    w_p = nc.dram_tensor("w_p", [CG, C], BF16, kind="ExternalInput")
    b_out = nc.dram_tensor("b_out", [128, OCC], F32, kind="ExternalInput")
    out_t = nc.dram_tensor("out_t", [C, T], F32, kind="ExternalOutput")
    with tile.TileContext(nc) as tc:
        _body(tc, T, x_t.ap(), w_qk.ap(), b_qk.ap(), w_v.ap(), ones_bf.ap(),
              w_p.ap(), b_out.ap(), out_t.ap())
    nc.compile()
    return nc


def shard_inputs(sequences, w_attn, b_attn, w_proj, b_proj):
    """Build the 8 per-core input maps. Core index = b*2 + g."""
    sequences = np.asarray(sequences, dtype=np.float32)
    w_attn = np.asarray(w_attn, dtype=np.float32)
    b_attn = np.asarray(b_attn, dtype=np.float32)
    w_proj = np.asarray(w_proj, dtype=np.float32)
    b_proj = np.asarray(b_proj, dtype=np.float32)
    B = sequences.shape[0]
    in_maps = []
    for b in range(B):
        for g in range(2):
            qs = slice(g * CG, (g + 1) * CG)
            ks = slice(C + g * CG, C + (g + 1) * CG)
            vs = slice(2 * C + g * CG, 2 * C + (g + 1) * CG)
            in_maps.append({
                "x_t": np.ascontiguousarray(sequences[b].T)
                    .astype(ml_dtypes.bfloat16),
                "w_qk": np.ascontiguousarray(
                    np.concatenate([w_attn[:, qs], w_attn[:, ks]], axis=1))
                    .astype(ml_dtypes.bfloat16),
                "b_qk": np.ascontiguousarray(
                    np.concatenate([b_attn[qs], b_attn[ks]])
                    .reshape(8, 128).T),
                "w_v": np.ascontiguousarray(w_attn[:, vs])
                    .astype(ml_dtypes.bfloat16),
                "ones_bf": np.ones((128, CG), ml_dtypes.bfloat16),
                "w_p": np.ascontiguousarray(w_proj[g * CG:(g + 1) * CG, :])
                    .astype(ml_dtypes.bfloat16),
                # softmax rows sum to 1, so the v-bias folds into the output
                # bias: y_g = attn@(x@w_v) @ w_p + (b_v@w_p [+ b_proj on g0])
                "b_out": np.ascontiguousarray(
                    (b_attn[vs] @ w_proj[g * CG:(g + 1) * CG, :]
                     + (b_proj if g == 0 else 0.0))
                    .astype(np.float32).reshape(8, 128).T),
            })
    return in_maps


def unshard_outputs(outs, B, T):
    """outs: list of 8 [C, T] partials, core index = b*2+g."""
    y = np.empty((B, T, C), np.float32)
    for b in range(B):
        y[b] = (outs[2 * b] + outs[2 * b + 1]).T
    return y


_NC_CACHE = {}


def kernel(sequences, w_attn, b_attn, w_proj, b_proj):
    sequences = np.asarray(sequences, dtype=np.float32)
    B, T, _ = sequences.shape
    in_maps = shard_inputs(sequences, w_attn, b_attn, w_proj, b_proj)
    if T not in _NC_CACHE:
        _NC_CACHE[T] = build_nc(T)
    nc = _NC_CACHE[T]
    res = run_bass_kernel_spmd(nc, in_maps, list(range(N_CORES)))
    outs = [res.results[i]["out_t"] for i in range(N_CORES)]
    return unshard_outputs(outs, B, T)


if __name__ == "__main__":
    rng = np.random.default_rng(0)
    B, T = 4, 2048
    seq = rng.standard_normal((B, T, C), dtype=np.float32)
    wa = rng.standard_normal((C, 3 * C), dtype=np.float32) / np.sqrt(C)
    ba = np.zeros(3 * C, np.float32)
    wp = rng.standard_normal((C, C), dtype=np.float32) / np.sqrt(C)
    bp = np.zeros(C, np.float32)
    y = kernel(seq, wa, ba, wp, bp)
    print(y.shape, y.dtype)
